# revision 29
# baseline (speedup 1.0000x reference)
"""Block-local self-attention (BLOCK=128, 3-block windows + global token) on 8
Trainium2 NeuronCores.

Sharding: batch*heads = 32 (n,h) pairs -> 4 pairs per core, no cross-core comms.

Per-core device kernel, per pair (all heavy O(T*window) work):
  - QK: for each k-block j (32), one matmul scoresT[k in j, q in blocks
    qlo..qlo+2] = K_j^T Q (stationary = K_j [65,128] incl. a mask row,
    moving = a contiguous [65,384] slice of the natural Q^T layout; the
    1/sqrt(d) scale is folded into Q on the host, the additive mask rides
    as a 65th contraction row).  3 slabs share one [128,1536] PSUM tile.
  - exp on ScalarE: one ACTIVATE per 3-slab batch, PSUM->SBUF bf16.
  - PV transposed: stationary = V'_j [128,65] ([V | ones] block; the ones
    column accumulates the softmax denominator), moving = 128-wide exp
    slices -> ctxT[d, q] accumulated in PSUM.  4 windows share one PSUM
    bank ([65,512]); window 4g's first matmul opens the bank with
    start=True (the whole-bank has_written clear happens before any other
    window touches the bank, and later windows' first writes land on
    cleared bits = overwrite), so no separate bank-clear is needed.
  - DVE copies each closed ctxT bank to an SBUF out tile; 2 DMAs/pair.

The batch pipeline is flattened across the 4 pairs (QK two batches ahead,
exp one ahead of PV) so no engine drains at pair boundaries.

Host side (O(T*D) only): input packing, the global-token rank-1 slot
(e0 = exp(q . k0)), the global query row (token 0 attends to all keys),
and the final division by the denominator row.
"""

import numpy as np
import ml_dtypes

N, H, T, D = 2, 16, 4000, 64
BLOCK = 128
TP = 4096            # padded token count (32 blocks)
W = 32               # number of 128-blocks
NCORES = 8
PAIRS = N * H        # 32
PPC = PAIRS // NCORES  # pairs per core
NEG = -70.0          # masked-key offset; small enough that the Schraudolph
                     # affine stays in positive int16 range (exp(-61)~1e-27
                     # is a negligible additive residue vs. O(100) denoms)
SCALE = 1.0 / np.sqrt(np.float32(D))
BQ = 3               # slabs (k-blocks) per QK/exp batch
SCH_A = 128.0 / float(np.log(2.0))   # bf16-Schraudolph exp: bitcast(int16(
SCH_B = 16248.5                      #   round(x*SCH_A + SCH_B))) ~= e^x

_prog_cache = {}


def _qlo(j):
    return min(max(j - 1, 0), W - 3)


def _batches():
    out, j = [], 0
    while j < W:
        out.append(list(range(j, min(j + BQ, W))))
        j += BQ
    return out


def _build_program():
    if "nc" in _prog_cache:
        return _prog_cache["nc"]

    import concourse.bacc as bacc
    import concourse.mybir as mybir
    from concourse import tile

    dt = mybir.dt
    EXP = mybir.ActivationFunctionType.Exp

    nc = bacc.Bacc("TRN2", target_bir_lowering=False, debug=False,
                   num_devices=NCORES)
    qts_d = nc.dram_tensor("qts", [PPC, 65, TP], dt.bfloat16,
                           kind="ExternalInput").ap()
    kte_d = nc.dram_tensor("kte", [PPC, 65, TP], dt.bfloat16,
                           kind="ExternalInput").ap()
    vp_d = nc.dram_tensor("vp", [PPC, 128, W * 65], dt.bfloat16,
                          kind="ExternalInput").ap()
    out_d = nc.dram_tensor("out", [PPC, 65, TP], dt.bfloat16,
                           kind="ExternalOutput").ap()

    pair_batches = _batches()          # per-pair batch list (slab indices)
    NPB = len(pair_batches)
    # global flattened batch list: (pair, slabs)
    gbatches = [(p, sl) for p in range(PPC) for sl in pair_batches]
    NB = len(gbatches)

    with tile.TileContext(nc) as tc:
        with (
            tc.tile_pool(name="qts", bufs=2) as qts_pool,
            tc.tile_pool(name="kte", bufs=2) as kte_pool,
            tc.tile_pool(name="vp", bufs=2) as vp_pool,
            tc.tile_pool(name="ex", bufs=3) as ex_pool,
            tc.tile_pool(name="small", bufs=1) as small_pool,
            tc.tile_pool(name="outp", bufs=2) as out_pool,
            tc.tile_pool(name="sc", bufs=2, space="PSUM") as sc_pool,
            tc.tile_pool(name="ctx", bufs=2, space="PSUM") as ctx_pool,
        ):
            def load_pair(p):
                # chunked so the first QK only waits on the head of the
                # stream (subtile deps), and spread across the Sync and
                # GpSimd HWDGE rings (descriptor issue is ~900ns each; the
                # Scalar ring is reserved for the bottleneck ACT queue)
                kte_t = kte_pool.tile([65, TP], dt.bfloat16, tag="kte",
                                      name=f"kte_{p}")
                qts_t = qts_pool.tile([65, TP], dt.bfloat16, tag="qts",
                                      name=f"qts_{p}")
                vp_t = vp_pool.tile([128, W * 65], dt.bfloat16, tag="vp",
                                    name=f"vp_{p}")
                # pair 0 is latency-critical (nothing to overlap with), so
                # its stream is chunked finely, with the first vp chunk
                # right behind kte's head (each ring only keeps ~2 transfers
                # in flight, so issue order = completion order).  Later
                # pairs prefetch a full pair ahead; fewer, bigger chunks.
                # batch b consumes kte cols <= (3b+3)*128, qts <= (3b+5)*128
                cuts = (0, 512, 1280, 2560, TP) if p == 0 else (0, 2048, TP)
                first = True
                for a, b in zip(cuts, cuts[1:]):
                    nc.sync.dma_start(kte_t[:, a:b], kte_d[p, :, a:b])
                    if first:
                        nc.sync.dma_start(vp_t[:, 0:1040], vp_d[p, :, 0:1040])
                        first = False
                    nc.gpsimd.dma_start(qts_t[:, a:b], qts_d[p, :, a:b])
                nc.gpsimd.dma_start(vp_t[:, 1040:W * 65],
                                    vp_d[p, :, 1040:W * 65])
                return qts_t, kte_t, vp_t

            # PE warm-up: dense 128-row matmuls (1-row weights do NOT
            # register on the HAM activity monitor) bridge the gap until the
            # first pair's inputs land, keeping the clock gate open.
            warm_sb = small_pool.tile([128, 512], dt.bfloat16, tag="warm")
            nc.vector.memset(warm_sb[:], 0.25)
            # preload the ACT exp table (~1.5us) during the DMA wait
            warm_ex = small_pool.tile([1, 1], dt.bfloat16, tag="wex")
            nc.scalar.activation(warm_ex[:], warm_sb[0:1, 0:1], EXP)
            warm_ps = sc_pool.tile([128, 512], dt.float32, tag="sc",
                                   name="warm_ps")
            for r in range(13):
                nc.tensor.matmul(warm_ps[:], warm_sb[:, 0:128],
                                 warm_sb[:], start=True, stop=True)

            pending = {0: load_pair(0)}
            tiles = {}                  # pair -> (qts_t, kte_t, vp_t)
            outts = {}                  # pair -> out tile
            ex_tiles = {}               # global batch idx -> ex tile
            ctx_tiles = {}              # (pair, group) -> psum tile
            slab_gb = {}                # (pair, slab) -> global batch idx
            for gb, (p, sl) in enumerate(gbatches):
                for j in sl:
                    slab_gb[(p, j)] = gb

            def get_pair(p):
                if p not in tiles:
                    tiles[p] = pending.pop(p)
                    if p + 1 < PPC:
                        pending[p + 1] = load_pair(p + 1)
                    outts[p] = out_pool.tile([65, TP], dt.bfloat16, tag="out",
                                             name=f"out_{p}")
                return tiles[p]

            def emit_qk(gb):
                p, sl = gbatches[gb]
                qts_t, kte_t, _ = get_pair(p)
                sc = sc_pool.tile([128, BQ * 512], dt.float32, tag="sc",
                                  name=f"sc_{p}_{gb}")
                for i, j in enumerate(sl):
                    c0 = _qlo(j) * 128
                    nc.tensor.matmul(
                        sc[:, i * 512:i * 512 + 384],
                        kte_t[:, j * 128:(j + 1) * 128],
                        qts_t[:, c0:c0 + 384],
                        start=True, stop=True)
                return sc

            def emit_exp(gb, sc):
                # the batch's last slab runs a one-pass Schraudolph exp on
                # the Vector engine (affine to int16, bit-viewed as bf16) to
                # offload the bottleneck ScalarE; ScalarE does the rest with
                # the exact LUT exp.  Both read disjoint PSUM banks.
                p, sl = gbatches[gb]
                nb = len(sl)
                na = nb - 1 if nb == BQ else nb   # offload only full batches
                ex = ex_pool.tile([128, BQ * 384], dt.bfloat16, tag="ex",
                                  name=f"ex_{p}_{gb}")
                nc.scalar.activation(
                    ex[:, 0:na * 384].rearrange("p (b x) -> p b x", x=384),
                    sc[:].rearrange("p (b x) -> p b x", x=512)[:, 0:na, 0:384],
                    EXP)
                if na < nb:
                    nc.vector.tensor_scalar(
                        ex[:, na * 384:nb * 384].bitcast(dt.int16),
                        sc[:, na * 512:na * 512 + 384], SCH_A, SCH_B,
                        mybir.AluOpType.mult, mybir.AluOpType.add)
                ex_tiles[gb] = ex

            def emit_pv(gb):
                p, sl = gbatches[gb]
                _, _, vp_t = get_pair(p)
                outt = outts[p]
                for j in sl:
                    for w in (j - 1, j, j + 1):
                        if not (0 <= w < W):
                            continue
                        g, wi = w // 4, w % 4
                        key = (p, g)
                        if key not in ctx_tiles:
                            ctx_tiles[key] = ctx_pool.tile(
                                [65, 512], dt.float32, tag="ctx",
                                name=f"ct_{p}_{g}")
                        ct = ctx_tiles[key]
                        gsl = w - _qlo(j)
                        bb = slab_gb[(p, j)]
                        exm = ex_tiles[bb]
                        off = (j - gbatches[bb][1][0]) * 384 + gsl * 128
                        # window 4g opens its bank: start=True clears the
                        # whole bank's has_written before any other window
                        # in the bank has written (slab-major order)
                        st = (wi == 0) and (j == max(w - 1, 0))
                        sp = (j == min(w + 1, W - 1))
                        nc.tensor.matmul(
                            ct[:, wi * 128:(wi + 1) * 128],
                            vp_t[:, j * 65:(j + 1) * 65],
                            exm[:, off:off + 128],
                            start=st, stop=sp,
                            skip_group_check=True)
                    # close groups whose last contributor is slab j; ship
                    # every 2 groups on the Vector ring (DVE just wrote it)
                    for g in range(8):
                        if (p, g) in ctx_tiles and min(4 * g + 4, W - 1) == j:
                            ct = ctx_tiles.pop((p, g))
                            if g == 7:
                                # the pair's last copy rides ScalarE: at the
                                # boundary the DVE is congested (group 6+7
                                # copies + next pair's schraud) while the
                                # ACT queue has slack
                                nc.scalar.copy(
                                    outt[:, g * 512:(g + 1) * 512], ct[:])
                            else:
                                nc.vector.tensor_scalar_mul(
                                    outt[:, g * 512:(g + 1) * 512], ct[:],
                                    1.0)
                            if p == PPC - 1:
                                # last pair: ship per-group so the final
                                # DMA isn't serialized behind the last copy
                                c0 = g * 512
                                nc.sync.dma_start(
                                    out_d[p, :, c0:c0 + 512],
                                    outt[:, c0:c0 + 512])
                            elif g % 2 == 1:
                                c0 = (g - 1) * 512
                                nc.sync.dma_start(
                                    out_d[p, :, c0:c0 + 1024],
                                    outt[:, c0:c0 + 1024])

            # software pipeline over the flattened batch list: QK two
            # batches ahead, exp one ahead of the PV consumption; PV before
            # the next QK so the PE never queues behind an exp it doesn't
            # depend on.
            scs = {0: emit_qk(0), 1: emit_qk(1)}
            for gb in range(NB):
                emit_exp(gb, scs.pop(gb))
                if gb >= 1:
                    emit_pv(gb - 1)
                if gb + 2 < NB:
                    scs[gb + 2] = emit_qk(gb + 2)
            emit_pv(NB - 1)

    nc.compile()
    _prog_cache["nc"] = nc
    return nc


def _prep_core_inputs(q, k, v, mask):
    """q,k,v: (PAIRS, T, D) f32; mask: (N, T) f32.  Returns per-core input
    dicts (bf16 device layouts)."""
    bf16 = ml_dtypes.bfloat16
    mpair = np.repeat(mask, H, axis=0)              # (PAIRS, T)

    qt = np.zeros((PAIRS, 65, TP), np.float32)
    qt[:, :D, :T] = q.transpose(0, 2, 1) * SCALE
    qt[:, D, :] = 1.0

    kt = np.zeros((PAIRS, 65, TP), np.float32)
    kt[:, :D, :T] = k.transpose(0, 2, 1)
    kt[:, D, :T] = mpair
    kt[:, D, T:] = NEG
    kt[:, D, 0] = NEG                               # k0 served by global slot

    vp = np.zeros((PAIRS, TP, 65), np.float32)
    vp[:, :T, :D] = v
    vp[:, :, D] = 1.0
    vp = vp.reshape(PAIRS, W, 128, 65).transpose(0, 2, 1, 3) \
           .reshape(PAIRS, 128, W * 65)

    qt = qt.astype(bf16)
    kt = kt.astype(bf16)
    vp = vp.astype(bf16)
    in_maps = []
    for c in range(NCORES):
        s = slice(c * PPC, (c + 1) * PPC)
        in_maps.append({"qts": qt[s], "kte": kt[s], "vp": vp[s]})
    return in_maps


def _host_global(q, k, v, mask):
    """e0 (token-0 key slot, per query) and the global query row, in f32."""
    mpair = np.repeat(mask, H, axis=0)              # (PAIRS, T)
    k0 = k[:, 0, :]                                 # (PAIRS, D)
    s0 = np.einsum('ptd,pd->pt', q, k0) * SCALE + mpair[:, 0:1]
    e0 = np.exp(s0)                                 # (PAIRS, T)

    q0 = q[:, 0, :]                                 # (PAIRS, D)
    gs = np.einsum('pd,ptd->pt', q0, k) * SCALE + mpair
    gs -= gs.max(axis=1, keepdims=True)
    ge = np.exp(gs)
    grow = np.einsum('pt,ptd->pd', ge, v) / ge.sum(axis=1, keepdims=True)
    return e0, grow


def _unshard(results, e0, grow, v0):
    o = np.concatenate([r["out"] for r in results], axis=0) \
          .astype(np.float32)                       # (PAIRS, 65, TP)
    ctx = o[:, :D, :T]                              # (PAIRS, D, T)
    den = o[:, D, :T] + e0                          # (PAIRS, T)
    ctx = ctx + e0[:, None, :] * v0[:, :, None]
    out = (ctx / den[:, None, :]).transpose(0, 2, 1)  # (PAIRS, T, D)
    out[:, 0, :] = grow
    return np.ascontiguousarray(out.reshape(N, H, T, D), dtype=np.float32)


def _run(inputs, trace=False, tmpdir=None):
    from concourse.bass_utils import run_bass_kernel_spmd

    q = np.asarray(inputs["query_layer"], np.float32).reshape(PAIRS, T, D)
    k = np.asarray(inputs["key_layer"], np.float32).reshape(PAIRS, T, D)
    v = np.asarray(inputs["value_layer"], np.float32).reshape(PAIRS, T, D)
    mask = np.asarray(inputs["attention_mask"], np.float32).reshape(N, T)

    nc = _build_program()
    in_maps = _prep_core_inputs(q, k, v, mask)
    e0, grow = _host_global(q, k, v, mask)
    res = run_bass_kernel_spmd(nc, in_maps, list(range(NCORES)),
                               trace=trace, tmpdir=tmpdir)
    return _unshard(res.results, e0, grow, v[:, 0, :]), res


def kernel(query_layer, key_layer, value_layer, attention_mask):
    out, _ = _run({
        "query_layer": query_layer,
        "key_layer": key_layer,
        "value_layer": value_layer,
        "attention_mask": attention_mask,
    })
    return out


# revision 30
# speedup vs baseline: 1.0084x; 1.0084x over previous
"""Block-local self-attention (BLOCK=128, 3-block windows + global token) on 8
Trainium2 NeuronCores.

Sharding: batch*heads = 32 (n,h) pairs -> 4 pairs per core, no cross-core comms.

Per-core device kernel, per pair (all heavy O(T*window) work):
  - QK: for each k-block j (32), one matmul scoresT[k in j, q in blocks
    qlo..qlo+2] = K_j^T Q (stationary = K_j [65,128] incl. a mask row,
    moving = a contiguous [65,384] slice of the natural Q^T layout; the
    1/sqrt(d) scale is folded into Q on the host, the additive mask rides
    as a 65th contraction row).  3 slabs share one [128,1536] PSUM tile.
  - exp on ScalarE: one ACTIVATE per 3-slab batch, PSUM->SBUF bf16.
  - PV transposed: stationary = V'_j [128,65] ([V | ones] block; the ones
    column accumulates the softmax denominator), moving = 128-wide exp
    slices -> ctxT[d, q] accumulated in PSUM.  4 windows share one PSUM
    bank ([65,512]); window 4g's first matmul opens the bank with
    start=True (the whole-bank has_written clear happens before any other
    window touches the bank, and later windows' first writes land on
    cleared bits = overwrite), so no separate bank-clear is needed.
  - DVE copies each closed ctxT bank to an SBUF out tile; 2 DMAs/pair.

The batch pipeline is flattened across the 4 pairs (QK two batches ahead,
exp one ahead of PV) so no engine drains at pair boundaries.

Host side (O(T*D) only): input packing, the global-token rank-1 slot
(e0 = exp(q . k0)), the global query row (token 0 attends to all keys),
and the final division by the denominator row.
"""

import numpy as np
import ml_dtypes

N, H, T, D = 2, 16, 4000, 64
BLOCK = 128
TP = 4096            # padded token count (32 blocks)
W = 32               # number of 128-blocks
NCORES = 8
PAIRS = N * H        # 32
PPC = PAIRS // NCORES  # pairs per core
NEG = -70.0          # masked-key offset; small enough that the Schraudolph
                     # affine stays in positive int16 range (exp(-61)~1e-27
                     # is a negligible additive residue vs. O(100) denoms)
SCALE = 1.0 / np.sqrt(np.float32(D))
BQ = 3               # slabs (k-blocks) per QK/exp batch
SCH_A = 128.0 / float(np.log(2.0))   # bf16-Schraudolph exp: bitcast(int16(
SCH_B = 16248.5                      #   round(x*SCH_A + SCH_B))) ~= e^x

_prog_cache = {}


def _qlo(j):
    return min(max(j - 1, 0), W - 3)


def _batches():
    out, j = [], 0
    while j < W:
        out.append(list(range(j, min(j + BQ, W))))
        j += BQ
    return out


def _build_program():
    if "nc" in _prog_cache:
        return _prog_cache["nc"]

    import concourse.bacc as bacc
    import concourse.mybir as mybir
    from concourse import tile

    dt = mybir.dt
    EXP = mybir.ActivationFunctionType.Exp

    nc = bacc.Bacc("TRN2", target_bir_lowering=False, debug=False,
                   num_devices=NCORES)
    qts_d = nc.dram_tensor("qts", [PPC, 65, TP], dt.bfloat16,
                           kind="ExternalInput").ap()
    kte_d = nc.dram_tensor("kte", [PPC, 65, TP], dt.bfloat16,
                           kind="ExternalInput").ap()
    vp_d = nc.dram_tensor("vp", [PPC, 128, W * 65], dt.bfloat16,
                          kind="ExternalInput").ap()
    out_d = nc.dram_tensor("out", [PPC, 65, TP], dt.bfloat16,
                           kind="ExternalOutput").ap()

    pair_batches = _batches()          # per-pair batch list (slab indices)
    NPB = len(pair_batches)
    # global flattened batch list: (pair, slabs)
    gbatches = [(p, sl) for p in range(PPC) for sl in pair_batches]
    NB = len(gbatches)

    with tile.TileContext(nc) as tc:
        with (
            tc.tile_pool(name="qts", bufs=2) as qts_pool,
            tc.tile_pool(name="kte", bufs=2) as kte_pool,
            tc.tile_pool(name="vp", bufs=2) as vp_pool,
            tc.tile_pool(name="ex", bufs=3) as ex_pool,
            tc.tile_pool(name="small", bufs=1) as small_pool,
            tc.tile_pool(name="outp", bufs=2) as out_pool,
            tc.tile_pool(name="sc", bufs=2, space="PSUM") as sc_pool,
            tc.tile_pool(name="ctx", bufs=2, space="PSUM") as ctx_pool,
        ):
            def load_pair(p):
                # chunked so the first QK only waits on the head of the
                # stream (subtile deps), and spread across the Sync and
                # GpSimd HWDGE rings (descriptor issue is ~900ns each; the
                # Scalar ring is reserved for the bottleneck ACT queue)
                kte_t = kte_pool.tile([65, TP], dt.bfloat16, tag="kte",
                                      name=f"kte_{p}")
                qts_t = qts_pool.tile([65, TP], dt.bfloat16, tag="qts",
                                      name=f"qts_{p}")
                vp_t = vp_pool.tile([128, W * 65], dt.bfloat16, tag="vp",
                                    name=f"vp_{p}")
                # pair 0 is latency-critical (nothing to overlap with), so
                # its stream is chunked finely, with the first vp chunk
                # right behind kte's head (each ring only keeps ~2 transfers
                # in flight, so issue order = completion order).  Later
                # pairs prefetch a full pair ahead; fewer, bigger chunks.
                # batch b consumes kte cols <= (3b+3)*128, qts <= (3b+5)*128
                cuts = (0, 512, 1024, 1536, 2048, 2560, TP) if p == 0 \
                    else (0, 2048, TP)
                first = True
                for a, b in zip(cuts, cuts[1:]):
                    nc.sync.dma_start(kte_t[:, a:b], kte_d[p, :, a:b])
                    if first:
                        nc.sync.dma_start(vp_t[:, 0:1040], vp_d[p, :, 0:1040])
                        first = False
                    nc.gpsimd.dma_start(qts_t[:, a:b], qts_d[p, :, a:b])
                nc.gpsimd.dma_start(vp_t[:, 1040:W * 65],
                                    vp_d[p, :, 1040:W * 65])
                return qts_t, kte_t, vp_t

            # PE warm-up: dense 128-row matmuls (1-row weights do NOT
            # register on the HAM activity monitor) bridge the gap until the
            # first pair's inputs land, keeping the clock gate open.
            warm_sb = small_pool.tile([128, 512], dt.bfloat16, tag="warm")
            nc.vector.memset(warm_sb[:], 0.25)
            # preload the ACT exp table (~1.5us) during the DMA wait
            warm_ex = small_pool.tile([1, 1], dt.bfloat16, tag="wex")
            nc.scalar.activation(warm_ex[:], warm_sb[0:1, 0:1], EXP)
            warm_ps = sc_pool.tile([128, 512], dt.float32, tag="sc",
                                   name="warm_ps")
            for r in range(13):
                nc.tensor.matmul(warm_ps[:], warm_sb[:, 0:128],
                                 warm_sb[:], start=True, stop=True)

            pending = {0: load_pair(0)}
            tiles = {}                  # pair -> (qts_t, kte_t, vp_t)
            outts = {}                  # pair -> out tile
            ex_tiles = {}               # global batch idx -> ex tile
            ctx_tiles = {}              # (pair, group) -> psum tile
            slab_gb = {}                # (pair, slab) -> global batch idx
            for gb, (p, sl) in enumerate(gbatches):
                for j in sl:
                    slab_gb[(p, j)] = gb

            def get_pair(p):
                if p not in tiles:
                    tiles[p] = pending.pop(p)
                    if p + 1 < PPC:
                        pending[p + 1] = load_pair(p + 1)
                    outts[p] = out_pool.tile([65, TP], dt.bfloat16, tag="out",
                                             name=f"out_{p}")
                return tiles[p]

            def emit_qk(gb):
                p, sl = gbatches[gb]
                qts_t, kte_t, _ = get_pair(p)
                sc = sc_pool.tile([128, BQ * 512], dt.float32, tag="sc",
                                  name=f"sc_{p}_{gb}")
                for i, j in enumerate(sl):
                    c0 = _qlo(j) * 128
                    nc.tensor.matmul(
                        sc[:, i * 512:i * 512 + 384],
                        kte_t[:, j * 128:(j + 1) * 128],
                        qts_t[:, c0:c0 + 384],
                        start=True, stop=True)
                return sc

            def emit_exp(gb, sc):
                # the batch's last slab runs a one-pass Schraudolph exp on
                # the Vector engine (affine to int16, bit-viewed as bf16) to
                # offload the bottleneck ScalarE; ScalarE does the rest with
                # the exact LUT exp.  Both read disjoint PSUM banks.
                p, sl = gbatches[gb]
                nb = len(sl)
                na = nb - 1 if nb == BQ else nb   # offload only full batches
                ex = ex_pool.tile([128, BQ * 384], dt.bfloat16, tag="ex",
                                  name=f"ex_{p}_{gb}")
                nc.scalar.activation(
                    ex[:, 0:na * 384].rearrange("p (b x) -> p b x", x=384),
                    sc[:].rearrange("p (b x) -> p b x", x=512)[:, 0:na, 0:384],
                    EXP)
                if na < nb:
                    nc.vector.tensor_scalar(
                        ex[:, na * 384:nb * 384].bitcast(dt.int16),
                        sc[:, na * 512:na * 512 + 384], SCH_A, SCH_B,
                        mybir.AluOpType.mult, mybir.AluOpType.add)
                ex_tiles[gb] = ex

            def emit_pv(gb):
                p, sl = gbatches[gb]
                _, _, vp_t = get_pair(p)
                outt = outts[p]
                for j in sl:
                    for w in (j - 1, j, j + 1):
                        if not (0 <= w < W):
                            continue
                        g, wi = w // 4, w % 4
                        key = (p, g)
                        if key not in ctx_tiles:
                            ctx_tiles[key] = ctx_pool.tile(
                                [65, 512], dt.float32, tag="ctx",
                                name=f"ct_{p}_{g}")
                        ct = ctx_tiles[key]
                        gsl = w - _qlo(j)
                        bb = slab_gb[(p, j)]
                        exm = ex_tiles[bb]
                        off = (j - gbatches[bb][1][0]) * 384 + gsl * 128
                        # window 4g opens its bank: start=True clears the
                        # whole bank's has_written before any other window
                        # in the bank has written (slab-major order)
                        st = (wi == 0) and (j == max(w - 1, 0))
                        sp = (j == min(w + 1, W - 1))
                        nc.tensor.matmul(
                            ct[:, wi * 128:(wi + 1) * 128],
                            vp_t[:, j * 65:(j + 1) * 65],
                            exm[:, off:off + 128],
                            start=st, stop=sp,
                            skip_group_check=True)
                    # close groups whose last contributor is slab j; ship
                    # every 2 groups on the Vector ring (DVE just wrote it)
                    for g in range(8):
                        if (p, g) in ctx_tiles and min(4 * g + 4, W - 1) == j:
                            ct = ctx_tiles.pop((p, g))
                            if g == 7:
                                # the pair's last copy rides ScalarE: at the
                                # boundary the DVE is congested (group 6+7
                                # copies + next pair's schraud) while the
                                # ACT queue has slack
                                nc.scalar.copy(
                                    outt[:, g * 512:(g + 1) * 512], ct[:])
                            else:
                                nc.vector.tensor_scalar_mul(
                                    outt[:, g * 512:(g + 1) * 512], ct[:],
                                    1.0)
                            if p == PPC - 1:
                                # last pair: ship per-group so the final
                                # DMA isn't serialized behind the last copy
                                c0 = g * 512
                                nc.sync.dma_start(
                                    out_d[p, :, c0:c0 + 512],
                                    outt[:, c0:c0 + 512])
                            elif g % 2 == 1:
                                c0 = (g - 1) * 512
                                nc.sync.dma_start(
                                    out_d[p, :, c0:c0 + 1024],
                                    outt[:, c0:c0 + 1024])

            # software pipeline over the flattened batch list: QK two
            # batches ahead, exp one ahead of the PV consumption; PV before
            # the next QK so the PE never queues behind an exp it doesn't
            # depend on.
            scs = {0: emit_qk(0), 1: emit_qk(1)}
            for gb in range(NB):
                emit_exp(gb, scs.pop(gb))
                if gb >= 1:
                    emit_pv(gb - 1)
                if gb + 2 < NB:
                    scs[gb + 2] = emit_qk(gb + 2)
            emit_pv(NB - 1)

    nc.compile()
    _prog_cache["nc"] = nc
    return nc


def _prep_core_inputs(q, k, v, mask):
    """q,k,v: (PAIRS, T, D) f32; mask: (N, T) f32.  Returns per-core input
    dicts (bf16 device layouts)."""
    bf16 = ml_dtypes.bfloat16
    mpair = np.repeat(mask, H, axis=0)              # (PAIRS, T)

    qt = np.zeros((PAIRS, 65, TP), np.float32)
    qt[:, :D, :T] = q.transpose(0, 2, 1) * SCALE
    qt[:, D, :] = 1.0

    kt = np.zeros((PAIRS, 65, TP), np.float32)
    kt[:, :D, :T] = k.transpose(0, 2, 1)
    kt[:, D, :T] = mpair
    kt[:, D, T:] = NEG
    kt[:, D, 0] = NEG                               # k0 served by global slot

    vp = np.zeros((PAIRS, TP, 65), np.float32)
    vp[:, :T, :D] = v
    vp[:, :, D] = 1.0
    vp = vp.reshape(PAIRS, W, 128, 65).transpose(0, 2, 1, 3) \
           .reshape(PAIRS, 128, W * 65)

    qt = qt.astype(bf16)
    kt = kt.astype(bf16)
    vp = vp.astype(bf16)
    in_maps = []
    for c in range(NCORES):
        s = slice(c * PPC, (c + 1) * PPC)
        in_maps.append({"qts": qt[s], "kte": kt[s], "vp": vp[s]})
    return in_maps


def _host_global(q, k, v, mask):
    """e0 (token-0 key slot, per query) and the global query row, in f32."""
    mpair = np.repeat(mask, H, axis=0)              # (PAIRS, T)
    k0 = k[:, 0, :]                                 # (PAIRS, D)
    s0 = np.einsum('ptd,pd->pt', q, k0) * SCALE + mpair[:, 0:1]
    e0 = np.exp(s0)                                 # (PAIRS, T)

    q0 = q[:, 0, :]                                 # (PAIRS, D)
    gs = np.einsum('pd,ptd->pt', q0, k) * SCALE + mpair
    gs -= gs.max(axis=1, keepdims=True)
    ge = np.exp(gs)
    grow = np.einsum('pt,ptd->pd', ge, v) / ge.sum(axis=1, keepdims=True)
    return e0, grow


def _unshard(results, e0, grow, v0):
    o = np.concatenate([r["out"] for r in results], axis=0) \
          .astype(np.float32)                       # (PAIRS, 65, TP)
    ctx = o[:, :D, :T]                              # (PAIRS, D, T)
    den = o[:, D, :T] + e0                          # (PAIRS, T)
    ctx = ctx + e0[:, None, :] * v0[:, :, None]
    out = (ctx / den[:, None, :]).transpose(0, 2, 1)  # (PAIRS, T, D)
    out[:, 0, :] = grow
    return np.ascontiguousarray(out.reshape(N, H, T, D), dtype=np.float32)


def _run(inputs, trace=False, tmpdir=None):
    from concourse.bass_utils import run_bass_kernel_spmd

    q = np.asarray(inputs["query_layer"], np.float32).reshape(PAIRS, T, D)
    k = np.asarray(inputs["key_layer"], np.float32).reshape(PAIRS, T, D)
    v = np.asarray(inputs["value_layer"], np.float32).reshape(PAIRS, T, D)
    mask = np.asarray(inputs["attention_mask"], np.float32).reshape(N, T)

    nc = _build_program()
    in_maps = _prep_core_inputs(q, k, v, mask)
    e0, grow = _host_global(q, k, v, mask)
    res = run_bass_kernel_spmd(nc, in_maps, list(range(NCORES)),
                               trace=trace, tmpdir=tmpdir)
    return _unshard(res.results, e0, grow, v[:, 0, :]), res


def kernel(query_layer, key_layer, value_layer, attention_mask):
    out, _ = _run({
        "query_layer": query_layer,
        "key_layer": key_layer,
        "value_layer": value_layer,
        "attention_mask": attention_mask,
    })
    return out
